# revision 1
# baseline (speedup 1.0000x reference)
"""Trainium2 Bass kernel for the mini-Mamba block (B=2, L=4096, D=128).

Sharding: 8 cores = 2 batches x 4 channel-groups (64 of ED=256 channels each).
Each core redundantly computes the shared front-end for its batch (fused
LN.LN, in_proj, depthwise conv, x_proj), then exclusively runs the selective
scan for its 64 channels — (e,n)-pairs on partitions, L on the free dim via
the hardware tensor_tensor_scan — plus partial out_proj; a 4-core AllReduce
combines partials, each core then runs the MLP tail redundantly. The host
reassembles per-batch outputs from cores 0 and 4.

Layout notes (hardware constraints): matmul requires lhsT/rhs to share a
base partition in {0,32,64} and PSUM outputs to start at {0,32,64}; two SBUF
inputs of a vector op must share a base partition, while outputs (and single
inputs) may be partition-shifted. Hence: scan tiles are (32 e x 4 n) so the
y-reduction emits 32-row blocks; dbc rows live at bases {0:Bm, 32:Cm, 64:dt}
of one tile; selector weights carry duplicated rows at base 32.

Per-core channel permutation puts this core's 64 channels at rows 0..63 so
the SPMD program is identical across cores; only the input data differs.
"""
import sys

for _p in ("/opt/trn_rl_repo",):
    if _p not in sys.path:
        sys.path.insert(0, _p)

import numpy as np
import concourse.bass as bass
import concourse.tile as tile
from concourse import mybir
from concourse.bass_utils import run_bass_kernel_spmd
from contextlib import ExitStack

AF = mybir.ActivationFunctionType
OP = mybir.AluOpType
F32 = mybir.dt.float32

B, D, ED, N, KC, R = 2, 128, 256, 16, 4, 8
H = 2 * D
EDL = ED // 4          # 64 channels per core
NQ = 4                 # n-quarters (4 n-values per scan tile)
EG = 2                 # e-groups of 32 channels
M40 = R + 2 * N
M72 = 72
EPS = 1e-5


# ---------------------------------------------------------------- wait fix
def _engine_nop(nc, eng):
    m = {
        mybir.EngineType.DVE: nc.vector,
        mybir.EngineType.Activation: nc.scalar,
        mybir.EngineType.PE: nc.tensor,
        mybir.EngineType.Pool: nc.gpsimd,
        mybir.EngineType.SP: nc.sync,
    }
    return m[eng].nop()


def _split_waits(nc, max_waits=1):
    """walrus here rejects compute instructions with >1 sem wait; hoist
    extras onto standalone same-engine nops placed just before."""
    scratch = nc.m.functions[0].blocks[-1]
    for fn in nc.m.functions:
        for bb in fn.blocks:
            lst = bb.instructions
            i = 0
            while i < len(lst):
                inst = lst[i]
                si = inst.sync_info
                if si is not None and len(si.on_wait) > max_waits:
                    waits = list(si.on_wait)
                    keep, extra = waits[-max_waits:], waits[:-max_waits]
                    for w in extra:
                        _engine_nop(nc, inst.engine)
                        nop = scratch.instructions.pop()
                        nop.sync_info = mybir.SyncInfo(on_wait=[w], on_update=[])
                        lst.insert(i, nop)
                        i += 1
                    inst.sync_info = mybir.SyncInfo(
                        on_wait=keep, on_update=list(si.on_update)
                    )
                i += 1


def _mm(nc, out_ps, lhsT, rhs, cols, rhs_off=0, out_off=0, start=True, stop=True):
    """Matmul streaming `cols` columns in <=512-col pieces."""
    c = 0
    while c < cols:
        w = min(512, cols - c)
        nc.tensor.matmul(
            out_ps[:, out_off + c : out_off + c + w],
            lhsT,
            rhs[:, rhs_off + c : rhs_off + c + w],
            start=start,
            stop=stop,
        )
        c += w


# ---------------------------------------------------------------- program
def build(L, reps=1):
    nc = bass.Bass("TRN2", target_bir_lowering=False, debug=False, num_devices=8)

    def din(name, shape):
        return nc.dram_tensor(name, shape, F32, kind="ExternalInput").ap()

    dr = {
        "xT": din("xT", [D, L]),
        "cenM": din("cenM", [D, D]),
        "onesD": din("onesD", [D, 1]),
        "ones1": din("ones1", [1, D]),
        "onesBC": din("onesBC", [D, D]),
        "n1c": din("n1c", [D, 1]),
        "n1c2": din("n1c2", [D, 1]),
        "epsb": din("epsb", [D, 1]),
        "wuT": din("wuT", [D, ED]),
        "ubias": din("ubias", [D, 2]),
        "wzT": din("wzT", [D, EDL]),
        "zbias": din("zbias", [EDL, 1]),
        "convD": din("convD", [D, 2 * KC * D]),
        "convb": din("convb", [D, 2]),
        "xprojT": din("xprojT", [D, 2 * M72]),
        "dtwT": din("dtwT", [D, EDL]),     # rows 64:72 hold dt_w.T
        "dtb": din("dtb", [EDL, 1]),
        "Acol": din("Acol", [128, EG * NQ]),
        "DpD": din("DpD", [EDL, 1]),
        "selE32": din("selE32", [EDL, 128]),   # rows 32:64 duplicate 0:32
        "selNQ": din("selNQ", [EDL, NQ * 128]),  # rows 32:48 duplicate 0:16
        "selY32": din("selY32", [128, 32]),
        "owT": din("owT", [EDL, D]),
        "fc1T": din("fc1T", [D, H]),
        "fc1b": din("fc1b", [D, 2]),
        "fc2T": din("fc2T", [D, 2 * D]),
        "fc2b": din("fc2b", [D, 1]),
    }
    out_d = nc.dram_tensor("outT", [D, L], F32, kind="ExternalOutput").ap()
    a_dram = nc.dram_tensor("a_stage", [D, L], F32)
    ar_dram = nc.dram_tensor("ar_stage", [D, L], F32)

    with tile.TileContext(nc) as tc, ExitStack() as ctx:
        const = ctx.enter_context(tc.tile_pool(name="const", bufs=1))
        cw = {}
        for name, d in dr.items():
            if name == "xT":
                continue
            t = const.tile(list(d.shape), F32, tag="c_" + name)
            nc.sync.dma_start(t[:], d)
            cw[name] = t
        for rep in range(reps):
            _one_pass(nc, tc, cw, L, dr["xT"], out_d, a_dram, ar_dram, rep)
    _split_waits(nc)
    return nc


def _one_pass(nc, tc, cw, L, xT_d, out_d, a_dram, ar_dram, rep):
    import os
    KSTAGE = int(os.environ.get("KSTAGE", "9"))
    KSUB = int(os.environ.get("KSUB", "9"))
    LH = L // 2
    CW = min(512, LH)       # column chunk
    NCH = L // CW
    LR = max(1, L // 128)   # cols per partition in row<->grid mapping
    PP = CW // LR           # partitions per chunk in that mapping
    sfx = f"_r{rep}"

    with ExitStack() as P:
        per = P.enter_context(tc.tile_pool(name="per" + sfx, bufs=1))
        ch2 = P.enter_context(tc.tile_pool(name="ch2" + sfx, bufs=2))

        xT = per.tile([D, L], F32, tag="xT")
        nc.sync.dma_start(xT[:], xT_d)

        # ======== fused LN1(norm1, const affine) o LN2(inner) ========
        # pass 1: var1(l) = mean_d((x - mean_d x)^2) -> [128, L/128] grid
        v128 = per.tile([128, LR], F32, tag="v128")
        with ExitStack() as PS:
            ps = PS.enter_context(tc.tile_pool(name="psf" + sfx, bufs=2,
                                               space="PSUM"))
            pvp = PS.enter_context(tc.tile_pool(name="pvp" + sfx, bufs=2,
                                                space="PSUM"))
            for cc in range(NCH):
                pa = ps.tile([128, CW], F32, tag="psA")
                _mm(nc, pa, cw["cenM"][:], xT, CW, rhs_off=cc * CW)
                sqc = ch2.tile([128, CW], F32, tag="chA")
                nc.scalar.activation(sqc[:], pa[:], AF.Square)
                pv = pvp.tile([1, CW], F32, tag="pv")
                _mm(nc, pv, cw["onesD"][:], sqc, CW)
                vrowc = ch2.tile([1, CW], F32, tag="vrowc")
                nc.scalar.activation(vrowc[:], pv[:], AF.Copy)
                nc.sync.dma_start(v128[cc * PP:(cc + 1) * PP, :],
                                  vrowc[0:1, :])
        # s = c*r1*rsqrt(c^2*var1*r1^2 + eps),  r1 = rsqrt(var1+eps)
        r1 = per.tile([128, LR], F32, tag="r1")
        nc.scalar.activation(r1[:], v128[:], AF.Ln, bias=cw["epsb"][:, 0:1])
        nc.scalar.activation(r1[:], r1[:], AF.Exp, scale=-0.5)
        t1 = per.tile([128, LR], F32, tag="t1")
        nc.vector.tensor_tensor(t1[:], v128[:], r1[:], OP.mult)
        nc.vector.tensor_tensor(t1[:], t1[:], r1[:], OP.mult)
        r2 = per.tile([128, LR], F32, tag="r2")
        nc.scalar.activation(r2[:], t1[:], AF.Ln, bias=cw["epsb"][:, 0:1],
                             scale=cw["n1c2"][:, 0:1])
        nc.scalar.activation(r2[:], r2[:], AF.Exp, scale=-0.5)
        s128 = per.tile([128, LR], F32, tag="s128")
        nc.vector.scalar_tensor_tensor(s128[:], r1[:], cw["n1c"][:, 0:1],
                                       r2[:], OP.mult, OP.mult)

        # pass 2 (recompute center) + in_proj, fused per chunk
        u_raw = [per.tile([128, L + 3], F32, tag=f"uraw{g}", name=f"uraw{g}" + sfx)
                 for g in range(2)]
        szR = [per.tile([EDL, LH], F32, tag=f"szR{hf}", name=f"szR{hf}" + sfx)
               for hf in range(2)]
        with ExitStack() as PS:
            ps = PS.enter_context(tc.tile_pool(name="psf2" + sfx, bufs=2,
                                               space="PSUM"))
            psz = PS.enter_context(tc.tile_pool(name="psz" + sfx, bufs=2,
                                                space="PSUM"))
            for g in range(2):
                nc.gpsimd.memset(u_raw[g][:, 0:3], 0.0)
            for cc in range(NCH):
                srowc = ch2.tile([1, CW], F32, tag="vrowc")
                nc.sync.dma_start(srowc[0:1, :],
                                  s128[cc * PP:(cc + 1) * PP, :])
                pa = ps.tile([128, CW], F32, tag="psA")
                _mm(nc, pa, cw["cenM"][:], xT, CW, rhs_off=cc * CW)
                pb = ps.tile([128, CW], F32, tag="psB")
                _mm(nc, pb, cw["ones1"][:], srowc, CW)
                sbc = ch2.tile([128, CW], F32, tag="chA")
                nc.scalar.activation(sbc[:], pb[:], AF.Copy)
                tN = ch2.tile([128, CW], F32, tag="chB")
                nc.vector.tensor_tensor(tN[:], sbc[:], pa[:], OP.mult)
                for g in range(2):
                    pg = ps.tile([128, CW], F32, tag="psB")
                    _mm(nc, pg, cw["wuT"][:, 128 * g:128 * (g + 1)], tN, CW)
                    nc.scalar.activation(
                        u_raw[g][:, 3 + cc * CW: 3 + (cc + 1) * CW], pg[:],
                        AF.Identity, bias=cw["ubias"][:, g:g + 1])
                hf = (cc * CW) // LH
                lc = (cc * CW) % LH
                pz = psz.tile([EDL, CW], F32, tag="psZ")
                _mm(nc, pz, cw["wzT"][:], tN, CW)
                nc.scalar.activation(szR[hf][:, lc:lc + CW], pz[:], AF.Silu,
                                     bias=cw["zbias"][:, 0:1])

        if KSTAGE <= 1:
            import os as _os
            probe = _os.environ.get("KPROBE1", "")
            nc.sync.dma_start(out_d, cw[probe][:, 0:L] if probe else xT[:])
            return
        # ======== depthwise causal conv + silu ; x_proj ; delta ========
        u = [per.tile([128, L], F32, tag=f"u{g}", name=f"u{g}" + sfx)
             for g in range(2)]
        dbcP = per.tile([128, L], F32, tag="dbcP")  # Bm@0, Cm@32, dt@64
        delta = per.tile([EDL, L], F32, tag="delta")
        with ExitStack() as PS:
            ps = PS.enter_context(tc.tile_pool(name="pcx" + sfx, bufs=2,
                                               space="PSUM"))
            for g in range(2):
                for cc in range(NCH):
                    pa = ps.tile([128, CW], F32, tag="psA")
                    for k in range(KC):
                        _mm(nc, pa, cw["convD"][:, (KC * g + k) * D:
                                                (KC * g + k + 1) * D],
                            u_raw[g], CW, rhs_off=cc * CW + k,
                            start=(k == 0), stop=(k == KC - 1))
                    nc.scalar.activation(
                        u[g][:, cc * CW:(cc + 1) * CW], pa[:], AF.Silu,
                        bias=cw["convb"][:, g:g + 1])
            for cc in range(NCH):
                pb = ps.tile([128, CW], F32, tag="psB")
                for j in range(0, CW, 512):
                    w = min(512, CW - j)
                    for g in range(2):
                        nc.tensor.matmul(
                            pb[0:M72, j:j + w],
                            cw["xprojT"][:, M72 * g:M72 * (g + 1)],
                            u[g][:, cc * CW + j:cc * CW + j + w],
                            start=(g == 0), stop=(g == 1))
                cs = slice(cc * CW, (cc + 1) * CW)
                nc.scalar.activation(dbcP[0:M72, cs], pb[0:M72, :], AF.Copy)
            for cc in range(NCH):
                pa = ps.tile([128, CW], F32, tag="psA")
                _mm(nc, pa[0:EDL, :], cw["dtwT"][64:64 + R, :],
                    dbcP[64:64 + R, :], CW, rhs_off=cc * CW)
                spe = ch2.tile([EDL, CW], F32, tag="spe")
                nc.scalar.activation(spe[:], pa[0:EDL, :], AF.Exp,
                                     bias=cw["dtb"][0:EDL, 0:1])
                nc.vector.tensor_scalar(spe[:], spe[:], 1.0, 0.0, OP.add,
                                        OP.bypass)
                nc.scalar.activation(delta[:, cc * CW:(cc + 1) * CW],
                                     spe[:], AF.Ln)

        if KSTAGE <= 2:
            nc.sync.dma_start(out_d, u_raw[0][:, 3:3 + L] if int(__import__('os').environ.get('KPROBE','0')) else u[0][:, 0:L])
            return
        # ======== selective scan: (32e x 4n) tiles, p = 32*nl + e ========
        aT = per.tile([D, L], F32, tag="uraw0")
        hlast = per.tile([128, EG * NQ], F32, tag="hlast")
        for hf in range(2):
            beta = per.tile([EDL, LH], F32, tag="beta")
            nc.vector.tensor_tensor(beta[:], delta[:, hf * LH:(hf + 1) * LH],
                                    u[0][0:EDL, hf * LH:(hf + 1) * LH], OP.mult)
            yS = per.tile([EDL, LH], F32, tag="yS")
            for eg in range(EG):
                egs = slice(32 * eg, 32 * eg + 32)
                with ExitStack() as PS:
                    psy = PS.enter_context(tc.tile_pool(
                        name=f"psy{hf}{eg}" + sfx, bufs=1, space="PSUM"))
                    psc = PS.enter_context(tc.tile_pool(
                        name=f"psc{hf}{eg}" + sfx, bufs=2, space="PSUM"))
                    sbp = PS.enter_context(tc.tile_pool(
                        name=f"sbp{hf}{eg}" + sfx, bufs=2))
                    Y = psy.tile([32, LH], F32)
                    for nq in range(NQ):
                        ti = eg * NQ + nq
                        BmR = per.tile([128, LH], F32, tag="BmR")
                        CmR = per.tile([128, LH], F32, tag="u1")
                        for cc in range(LH // CW):
                            pa = psc.tile([128, CW], F32, tag="bA")
                            _mm(nc, pa, cw["selNQ"][0:N, nq * 128:(nq + 1) * 128],
                                dbcP[0:N, :], CW, rhs_off=hf * LH + cc * CW)
                            nc.scalar.activation(BmR[:, cc * CW:(cc + 1) * CW],
                                                 pa[:], AF.Copy)
                            pb = psc.tile([128, CW], F32, tag="bB")
                            _mm(nc, pb,
                                cw["selNQ"][32:32 + N, nq * 128:(nq + 1) * 128],
                                dbcP[32:32 + N, :], CW, rhs_off=hf * LH + cc * CW)
                            nc.scalar.activation(CmR[:, cc * CW:(cc + 1) * CW],
                                                 pb[:], AF.Copy)
                        hprev = None
                        for c in range(LH // CW):
                            lc = c * CW
                            if KSUB < 2:
                                continue
                            pa = psc.tile([128, CW], F32, tag="bA")
                            _mm(nc, pa, cw["selE32"][egs, :], delta[egs, :],
                                CW, rhs_off=hf * LH + lc)
                            dA = sbp.tile([128, CW], F32, tag="dA")
                            nc.scalar.activation(dA[:], pa[:], AF.Exp,
                                                 scale=cw["Acol"][:, ti:ti + 1])
                            if KSUB < 3:
                                continue
                            pb = psc.tile([128, CW], F32, tag="bB")
                            _mm(nc, pb, cw["selE32"][egs, :], beta[egs, :],
                                CW, rhs_off=lc)
                            dBu = sbp.tile([128, CW], F32, tag="dBu")
                            nc.vector.tensor_tensor(dBu[:], BmR[:, lc:lc + CW],
                                                    pb[:], OP.mult)
                            if KSUB < 4:
                                continue
                            h = sbp.tile([128, CW], F32, tag="h")
                            if hprev is None and hf == 0:
                                init = 0.0
                            elif hprev is None:
                                init = hlast[:, ti:ti + 1]
                            else:
                                init = hprev[:, CW - 1:CW]
                            nc.vector.tensor_tensor_scan(h[:], dA[:], dBu[:],
                                                         init, OP.mult, OP.add)
                            hprev = h
                            if KSUB < 5:
                                continue
                            hcm = sbp.tile([128, CW], F32, tag="hcm")
                            nc.vector.tensor_tensor(hcm[:], h[:],
                                                    CmR[:, lc:lc + CW], OP.mult)
                            if KSUB < 6:
                                continue
                            _mm(nc, Y[:, lc:lc + CW], cw["selY32"][:], hcm, CW,
                                start=(nq == 0), stop=False)
                        if hf == 0 and KSUB >= 4:
                            nc.vector.tensor_copy(hlast[:, ti:ti + 1],
                                                  hprev[:, CW - 1:CW])
                    if KSUB >= 6:
                        nc.scalar.activation(yS[egs, :], Y[:], AF.Copy)
                    else:
                        nc.gpsimd.memset(yS[egs, :], 0.0)
            # y = (scan_y + Dp*u) * silu(z); then out_proj partials
            yG = per.tile([EDL, LH], F32, tag="yG")
            nc.vector.scalar_tensor_tensor(
                yG[:], u[0][0:EDL, hf * LH:(hf + 1) * LH],
                cw["DpD"][:, 0:1], yS[:], OP.mult, OP.add)
            nc.vector.tensor_tensor(yG[:], yG[:], szR[hf][:], OP.mult)
            with ExitStack() as PS:
                ps = PS.enter_context(tc.tile_pool(name=f"pso{hf}" + sfx,
                                                   bufs=2, space="PSUM"))
                for cc in range(LH // CW):
                    pa = ps.tile([128, CW], F32, tag="psA")
                    _mm(nc, pa, cw["owT"][:], yG[:], CW, rhs_off=cc * CW)
                    nc.scalar.activation(
                        aT[:, hf * LH + cc * CW: hf * LH + (cc + 1) * CW],
                        pa[:], AF.Copy)

        if KSTAGE <= 3:
            nc.sync.dma_start(out_d, aT[:])
            return
        # ======== AllReduce over this batch's 4 cores ========
        nc.sync.dma_start(a_dram[:], aT[:])
        nc.gpsimd.collective_compute(
            "AllReduce", OP.add,
            replica_groups=[[0, 1, 2, 3], [4, 5, 6, 7]],
            ins=[a_dram[:]], outs=[ar_dram[:]])
        arT = per.tile([D, L], F32, tag="delta")
        nc.sync.dma_start(arT[:], ar_dram[:])

        if KSTAGE <= 4:
            nc.sync.dma_start(out_d, arT[:])
            return
        # ======== x_new = x + a ; LN(norm1, baked into fc1) ; MLP ========
        xn = per.tile([D, L], F32, tag="dbcP")
        nc.vector.tensor_tensor(xn[:], xT[:], arT[:], OP.add)
        hH = [per.tile([128, L], F32, tag=f"uraw{g}", name=f"hH{g}" + sfx)
              for g in range(2)]
        with ExitStack() as PS:
            ps = PS.enter_context(tc.tile_pool(name="psm0" + sfx, bufs=2,
                                               space="PSUM"))
            pvp = PS.enter_context(tc.tile_pool(name="pvm" + sfx, bufs=2,
                                                space="PSUM"))
            for cc in range(NCH):
                pa = ps.tile([128, CW], F32, tag="psA")
                _mm(nc, pa, cw["cenM"][:], xn, CW, rhs_off=cc * CW)
                sqc = ch2.tile([128, CW], F32, tag="chA")
                nc.scalar.activation(sqc[:], pa[:], AF.Square)
                pv = pvp.tile([1, CW], F32, tag="pv")
                _mm(nc, pv, cw["onesD"][:], sqc, CW)
                vrowc = ch2.tile([1, CW], F32, tag="vrowc")
                nc.scalar.activation(vrowc[:], pv[:], AF.Copy)
                nc.sync.dma_start(v128[cc * PP:(cc + 1) * PP, :],
                                  vrowc[0:1, :])
            nc.scalar.activation(s128[:], v128[:], AF.Ln,
                                 bias=cw["epsb"][:, 0:1])
            nc.scalar.activation(s128[:], s128[:], AF.Exp, scale=-0.5)
        with ExitStack() as PS:
            ps = PS.enter_context(tc.tile_pool(name="psm" + sfx, bufs=2,
                                               space="PSUM"))
            for cc in range(NCH):
                srowc = ch2.tile([1, CW], F32, tag="vrowc")
                nc.sync.dma_start(srowc[0:1, :],
                                  s128[cc * PP:(cc + 1) * PP, :])
                pa = ps.tile([128, CW], F32, tag="psA")
                _mm(nc, pa, cw["cenM"][:], xn, CW, rhs_off=cc * CW)
                pb = ps.tile([128, CW], F32, tag="psB")
                _mm(nc, pb, cw["ones1"][:], srowc, CW)
                sbc = ch2.tile([128, CW], F32, tag="chA")
                nc.scalar.activation(sbc[:], pb[:], AF.Copy)
                hN = ch2.tile([128, CW], F32, tag="chB")
                nc.vector.tensor_tensor(hN[:], sbc[:], pa[:], OP.mult)
                for g in range(2):
                    pg = ps.tile([128, CW], F32, tag="psB")
                    _mm(nc, pg, cw["fc1T"][:, 128 * g:128 * (g + 1)], hN, CW)
                    nc.scalar.activation(
                        hH[g][:, cc * CW:(cc + 1) * CW], pg[:], AF.Gelu,
                        bias=cw["fc1b"][:, g:g + 1])
            for cc in range(NCH):
                pb = ps.tile([128, CW], F32, tag="psB")
                for j in range(0, CW, 512):
                    w = min(512, CW - j)
                    for g in range(2):
                        nc.tensor.matmul(
                            pb[:, j:j + w],
                            cw["fc2T"][:, 128 * g:128 * (g + 1)],
                            hH[g][:, cc * CW + j:cc * CW + j + w],
                            start=(g == 0), stop=(g == 1))
                mlpc = ch2.tile([128, CW], F32, tag="chA")
                nc.scalar.activation(mlpc[:], pb[:], AF.Identity,
                                     bias=cw["fc2b"][:, 0:1])
                outc = ch2.tile([128, CW], F32, tag="chB")
                nc.vector.tensor_tensor(outc[:], xn[:, cc * CW:(cc + 1) * CW],
                                        mlpc[:], OP.add)
                nc.sync.dma_start(out_d[:, cc * CW:(cc + 1) * CW], outc[:])


# ---------------------------------------------------------------- host side
def make_in_maps(inputs, L):
    f32 = lambda k: np.asarray(inputs[k], np.float32)
    x = f32("x")
    norm1_g, norm1_b = f32("norm1_g"), f32("norm1_b")
    inner_g, inner_b = f32("inner_g"), f32("inner_b")
    in_w, conv_w, conv_b = f32("in_w"), f32("conv_w"), f32("conv_b")
    xproj_w, dt_w, dt_b = f32("xproj_w"), f32("dt_w"), f32("dt_b")
    A_log, Dp, out_w = f32("A_log"), f32("Dp"), f32("out_w")
    fc1_w, fc1_b, fc2_w, fc2_b = f32("fc1_w"), f32("fc1_b"), f32("fc2_w"), f32("fc2_b")

    assert np.ptp(norm1_g) == 0.0 and np.ptp(norm1_b) == 0.0, (
        "fused LN path requires a constant norm1 affine")
    c = float(norm1_g[0])

    cenM = (np.eye(D, dtype=np.float32) - np.float32(1.0 / D))
    onesD = np.full((D, 1), 1.0 / D, np.float32)
    ones1 = np.ones((1, D), np.float32)
    onesBC = np.full((D, D), 1.0 / D, np.float32)
    n1c = np.full((D, 1), c, np.float32)
    n1c2 = np.full((D, 1), c * c, np.float32)
    # scan tile partitions: p = 32*nl + e (nl 0..3, e 0..31)
    selE32 = np.zeros((EDL, 128), np.float32)
    for p in range(128):
        selE32[p % 32, p] = 1.0
        selE32[32 + p % 32, p] = 1.0
    selY32 = selE32[0:32].T.copy()
    selNQ = np.zeros((EDL, NQ * 128), np.float32)
    for nq in range(NQ):
        for p in range(128):
            selNQ[nq * 4 + p // 32, nq * 128 + p] = 1.0
            selNQ[32 + nq * 4 + p // 32, nq * 128 + p] = 1.0
    fc1T = np.ascontiguousarray((fc1_w * norm1_g[None, :]).T)
    fc1b = np.ascontiguousarray((fc1_b + fc1_w @ norm1_b).reshape(2, D).T)
    fc2T = np.ascontiguousarray(fc2_w.T.reshape(2, D, D)
                                .transpose(1, 0, 2).reshape(D, 2 * D))
    fc2b = fc2_b.reshape(D, 1)
    A_full = -np.exp(A_log)

    in_maps = []
    for core in range(8):
        b, g = core // 4, core % 4
        mine = np.arange(EDL * g, EDL * (g + 1))
        rest = np.setdiff1d(np.arange(ED), mine)
        perm = np.concatenate([mine, rest])

        u_rows = in_w[perm] * inner_g[None, :]
        ubias = np.ascontiguousarray((in_w[perm] @ inner_b).reshape(2, D).T)
        z_rows = in_w[ED + mine] * inner_g[None, :]
        zbias = (in_w[ED + mine] @ inner_b).reshape(EDL, 1)
        cwv = conv_w[perm, 0, :]
        convD = np.zeros((D, 2 * KC * D), np.float32)
        for grp in range(2):
            for k in range(KC):
                blk = np.diag(cwv[128 * grp:128 * (grp + 1), k])
                convD[:, (KC * grp + k) * D:(KC * grp + k + 1) * D] = blk
        convb = np.ascontiguousarray(conv_b[perm].reshape(2, D).T)
        xpT = xproj_w[:, perm].T          # [ED, 40] rows=e, cols=dt|B|C
        xpP = np.zeros((ED, 72), np.float32)
        xpP[:, 0:N] = xpT[:, R:R + N]          # Bm -> rows 0:16
        xpP[:, 32:32 + N] = xpT[:, R + N:M40]  # Cm -> rows 32:48
        xpP[:, 64:64 + R] = xpT[:, 0:R]        # dt -> rows 64:72
        xprojT = np.ascontiguousarray(
            xpP.reshape(2, D, 72).transpose(1, 0, 2).reshape(D, 2 * 72))
        dtwT = np.zeros((D, EDL), np.float32)
        dtwT[64:64 + R, :] = dt_w[mine].T
        dtb = dt_b[mine].reshape(EDL, 1)
        A = A_full[mine]          # [64, 16]
        Acol = np.zeros((128, EG * NQ), np.float32)
        for eg in range(EG):
            for nq in range(NQ):
                for p in range(128):
                    nl, e = p // 32, p % 32
                    Acol[p, eg * NQ + nq] = A[32 * eg + e, nq * 4 + nl]
        DpD = Dp[mine].reshape(EDL, 1).astype(np.float32)
        owT = np.ascontiguousarray(out_w[:, mine].T)

        in_maps.append({
            "xT": np.ascontiguousarray(x[b, :L].T),
            "cenM": cenM, "onesD": onesD, "ones1": ones1, "onesBC": onesBC,
            "n1c": n1c, "n1c2": n1c2, "epsb": np.full((D, 1), EPS, np.float32),
            "wuT": np.ascontiguousarray(u_rows.T), "ubias": ubias,
            "wzT": np.ascontiguousarray(z_rows.T), "zbias": zbias,
            "convD": convD, "convb": convb,
            "xprojT": xprojT, "dtwT": dtwT, "dtb": dtb,
            "Acol": Acol, "DpD": DpD,
            "selE32": selE32, "selNQ": selNQ, "selY32": selY32,
            "owT": owT,
            "fc1T": fc1T, "fc1b": fc1b, "fc2T": fc2T, "fc2b": fc2b,
        })
    return in_maps


_cache = {}


def run(inputs, L=4096, reps=1):
    key = (L, reps)
    if key not in _cache:
        _cache[key] = build(L, reps)
    nc = _cache[key]
    in_maps = make_in_maps(inputs, L)
    res = run_bass_kernel_spmd(nc, in_maps, list(range(8)))
    out = np.empty((B, L, D), np.float32)
    out[0] = res.results[0]["outT"].T
    out[1] = res.results[4]["outT"].T
    return out


def kernel(**inputs) -> np.ndarray:
    return run(inputs, L=4096)

